# revision 14
# baseline (speedup 1.0000x reference)
"""Distributed Trainium2 kernel for the audio-visual contrastive loss.

Math (reference):
    a = l2norm(audio)  (B=32, Na=512, D=768)
    v = l2norm(visual) (B=32, Nv=256, D=768)
    token_sims[b,c,n,m] = (a[b,n] . v[c,m]) / T
    clip_sims = mean_n max_m token_sims          (B, B)
    loss = mean_b -0.5*(log_softmax(clip)[b,b] + log_softmax(clip.T)[b,b])

Distribution over 8 NeuronCores:
    - audio batch is sharded 4 clips/core; visual batch is sharded 4 clips/core
      for the (normalize + transpose) prep, then AllGather'd as fp8 (d-major)
      in G=4 chunks (1 clip/core each) so the collectives pipeline with the
      main-loop matmuls.
    - RING-RELATIVE consumption: mx position q = g*8 + t where t is the ring
      step (t=0 = own clip).  Own clips come straight from local SBUF, so 96
      of the 768 matmuls run during the ~55us first-collective staging
      window; the other ranks are read from the gather output with a
      partition-id-relative dynamic row offset.
    - the loss is finished with per-core partial softmax stats and a single
      tiny AllReduce; per-core diag-mask and position-permutation inputs
      absorb all rank-dependence.
"""

import sys

for _p in ("/opt/trn_rl_repo",):
    if _p not in sys.path:
        sys.path.insert(0, _p)

import numpy as np

import concourse.bacc as bacc
import concourse.bass as bass_mod
import concourse.mybir as mybir
import concourse.tile as tile
from concourse.tile_rust import add_dep_helper

N_CORES = 8
B = 32
NA = 512
NV = 256
D = 768
TEMPERATURE = 0.1
BL = B // N_CORES            # 4 clips per core
AROWS = BL * NA              # 2048 audio rows per core
VROWS = BL * NV              # 1024 visual rows per core
KD = D // 128                # 6 contraction chunks
KD2 = KD // 2                # 3 DoubleRow chunk-pairs
NT_A = AROWS // 128          # 16 audio row-tiles
NT_V = VROWS // 128          # 8 visual row-tiles
G = 4                        # visual AllGather chunks (1 clip/core each)
VCH = VROWS // G             # 256 visual rows per chunk per core

F32 = mybir.dt.float32
BF16 = mybir.dt.bfloat16
FP8 = mybir.dt.float8e4
AX = mybir.AxisListType
ALU = mybir.AluOpType
ACT = mybir.ActivationFunctionType
SCL = 16.0                   # fp8 pre-scale (folded into the norm rsqrt)
SC = 1.0 / (NA * TEMPERATURE * SCL * SCL)   # psum-count -> clip_sims scale


def pos_of_clip(c, core):
    # ring-relative positions: clip c = 4*r + g sits at q = g*8 + t where
    # t = (r - core) mod 8 is the ring step (t=0 is the core's own clip).
    g = c % G
    r = c // BL
    t = (r - core) % N_CORES
    return g * 8 + t


def build():
    nc = bacc.Bacc("TRN2", target_bir_lowering=False, debug=False,
                   num_devices=N_CORES)
    a_in = nc.declare_dram_parameter("audio", [AROWS, D], F32, isOutput=False)
    v_in = nc.declare_dram_parameter("visual", [VROWS, D], F32,
                                     isOutput=False)
    dmask_in = nc.declare_dram_parameter("dmask", [1, 128], F32,
                                         isOutput=False)
    perm_in = nc.declare_dram_parameter("perm", [32, 32], F32,
                                        isOutput=False)
    out = nc.declare_dram_parameter("out", [1, 1], F32, isOutput=True)
    ident_dram = nc.inline_tensor(np.eye(128, dtype=np.float32), name="ident")
    rg = [list(range(N_CORES))]

    with tile.TileContext(nc) as tc:
        with (
            tc.tile_pool(name="persist", bufs=1) as pp,
            tc.tile_pool(name="work", bufs=3) as wp,
            tc.tile_pool(name="ps", bufs=2, space="PSUM") as ps,
            tc.tile_pool(name="dram", bufs=1, space="DRAM") as dp,
        ):
            # ---- constants ------------------------------------------------
            ident_f32 = pp.tile([128, 128], F32, tag="identf")
            nc.sync.dma_start(out=ident_f32[:], in_=ident_dram[:])
            ident_bf = pp.tile([128, 128], BF16, tag="identb")
            nc.scalar.copy(ident_bf[:], ident_f32[:])
            ones = pp.tile([128, 1], F32, tag="ones")
            nc.gpsimd.memset(ones[:], 1.0)
            # warm the ACT Ln/Exp tables off the critical path
            wrma = wp.tile([1, 1], F32, tag="wrma")
            nc.vector.memset(wrma[:], 1.0)
            wrmb = wp.tile([1, 1], F32, tag="wrmb")
            nc.scalar.activation(wrmb[:], wrma[:], ACT.Exp)
            wrmc = wp.tile([1, 1], F32, tag="wrmc")
            nc.scalar.activation(wrmc[:], wrmb[:], ACT.Ln)
            vec = pp.tile([1, 64], F32, tag="vec")
            nc.vector.memset(vec[:], 0.0)

            # ---- persistent tensors ---------------------------------------
            VTW = N_CORES * VROWS        # 8192 vT columns per d-chunk
            aTf = [pp.tile([128, 2 * AROWS], FP8, tag=f"aT8{k2}",
                           name=f"aT8{k2}") for k2 in range(KD2)]
            vstall = pp.tile([128, KD * VROWS], FP8, tag="vstall")
            vTf = [pp.tile([128, 2 * VTW], FP8, tag=f"vT8{k2}",
                           name=f"vT8{k2}") for k2 in range(KD2)]
            mx = pp.tile([128, 512], F32, tag="mx")

            # ---- row-tile prep --------------------------------------------
            class BatchRec:
                pass

            def prep_batch(src, t0, nb, dst_of_k, load_group):
                rec = BatchRec()
                raws = []
                ssb = wp.tile([128, nb], F32, tag="ssb", name="ssb", bufs=2)
                for j in range(nb):
                    t = t0 + j
                    raw = wp.tile([128, D], F32, tag="raw", name="raw",
                                  bufs=8)
                    load_group.append(
                        nc.sync.dma_start(out=raw[:],
                                          in_=src[t * 128:(t + 1) * 128, :]))
                    sq = nc.scalar.activation(
                        wp.tile([128, D], F32, tag="sqs", name="sqs",
                                bufs=2)[:],
                        raw[:], ACT.Square, accum_out=ssb[:, j:j + 1])
                    if j == 0:
                        rec.sq_first = sq
                    raws.append(raw)
                nrm = wp.tile([128, nb], F32, tag="nrm", name="nrm", bufs=2)
                nc.scalar.activation(nrm[:], ssb[:], ACT.Sqrt,
                                     scale=1.0 / (SCL * SCL))
                rnb = wp.tile([128, nb], F32, tag="rnb", name="rnb", bufs=2)
                nc.vector.reciprocal(rnb[:], nrm[:])
                nbfs = []
                for j in range(nb):
                    nbf = wp.tile([128, D], BF16, tag="nbf", name="nbf",
                                  bufs=6)
                    rec.cast_last = nc.scalar.activation(
                        nbf[:], raws[j][:], ACT.Copy, bias=0.0,
                        scale=rnb[:, j:j + 1])
                    nbfs.append(nbf)
                rec.tr_first = rec.tr_last = None
                rec.cp_first = rec.cp_last = None
                for j in range(nb):
                    for k in range(KD):
                        ptb = ps.tile([128, 128], BF16, tag="pt", name="ptb",
                                      bufs=3)
                        tr = nc.tensor.transpose(
                            ptb[:], nbfs[j][:, 128 * k:128 * (k + 1)],
                            ident_bf[:])
                        dst_tile, col = dst_of_k(t0, k)
                        cp = nc.vector.tensor_copy(
                            dst_tile[:, col + j * 128:col + (j + 1) * 128],
                            ptb[:])
                        if rec.tr_first is None:
                            rec.tr_first, rec.cp_first = tr, cp
                        rec.tr_last, rec.cp_last = tr, cp
                return rec

            # ---- visual prep + bounce + chunked AllGather -----------------
            vis_loads, aud_loads1, aud_loads2 = [], [], []
            vt_loads = []
            vgath = []
            vis_recs, aud_recs = [], []
            nbv = NT_V // G              # 2 tiles per chunk
            vst3 = vstall[:].rearrange("p (k c) -> p k c", k=KD)
            for g in range(G):
                vis_recs.append(prep_batch(
                    v_in, g * nbv, nbv,
                    lambda t0, k: (vstall, k * VROWS + (t0 // nbv) * VCH),
                    vis_loads))
                vb = dp.tile([128, KD * VCH // 4], F32, tag=f"vb{g}",
                             name=f"vb{g}")
                nc.scalar.dma_start(
                    out=vb[:, :].rearrange("p (k c) -> p k c", k=KD),
                    in_=vst3[:, :, g * VCH:(g + 1) * VCH].bitcast(F32))
                vg = dp.tile([N_CORES * 128, KD * VCH // 4], F32,
                             tag=f"vg{g}", name=f"vg{g}",
                             addr_space="Shared")
                nc.gpsimd.collective_compute(
                    "AllGather", ALU.bypass, replica_groups=rg,
                    ins=[vb[:, :].opt()], outs=[vg[:, :].opt()])
                vgath.append(vg)

            dmask = pp.tile([1, 128], F32, tag="dmask")
            vis_loads.append(nc.sync.dma_start(out=dmask[:], in_=dmask_in[:]))
            perm = pp.tile([32, 32], F32, tag="perm")
            vis_loads.append(nc.sync.dma_start(out=perm[:], in_=perm_in[:]))

            # ---- own clips: vstall -> vTf local blocks (DVE engine ops, ---
            # no DMA ring involved so the scheduler cannot push them behind
            # the gathers).  Block 14+pair holds (t=0 of chunks 2p, 2p+1).
            loc_first, loc_last = None, None
            for pair in range(2):
                fL = 14 + pair
                for k2 in range(KD2):
                    dst = vTf[k2][:, fL * 1024:(fL + 1) * 1024].bitcast(
                        F32).rearrange("p (ko h n) -> p ko h n", ko=2, h=2)
                    src = vst3[:, 2 * k2:2 * k2 + 2,
                               2 * pair * VCH:
                               (2 * pair + 2) * VCH].bitcast(F32).rearrange(
                        "p ko (h n) -> p ko h n", h=2)
                    cp = nc.vector.tensor_copy(dst, src)
                    if loc_first is None:
                        loc_first = cp
                    loc_last = cp

            # ---- audio prep ----------------------------------------------
            for t0 in range(0, NT_A, 8):
                aud_recs.append(prep_batch(
                    a_in, t0, 8,
                    lambda t0_, k: (aTf[k // 2],
                                    (k % 2) * AROWS + t0_ * 128),
                    aud_loads1 if t0 == 0 else aud_loads2))

            # ---- scheduler pins: visual prep, then local copies, then -----
            # audio prep, per engine.  Without these the scheduler hoists
            # audio prep ahead of the visual chain and the bounces (hence
            # the gathers) slip by ~20us.
            add_dep_helper(aud_recs[0].sq_first.ins,
                           vis_recs[-1].cast_last.ins, sync=False,
                           reason="act: visual prep first")
            add_dep_helper(aud_recs[0].tr_first.ins,
                           vis_recs[-1].tr_last.ins, sync=False,
                           reason="pe: visual prep first")
            add_dep_helper(loc_first.ins, vis_recs[-1].cp_last.ins,
                           sync=False, reason="dve: local after visual")
            add_dep_helper(aud_recs[0].cp_first.ins, loc_last.ins,
                           sync=False, reason="dve: audio after local")

            # ---- other ranks: gather output -> vTf, ring-relative ---------
            # dynamic row offset r = (pid+t)%8 selects the rank block.
            #   f = 3*g + u      (u=0..2): chunk g, ring steps (2u+1, 2u+2)
            #   f = 12 + pair    : ring step 7 of chunks (2*pair, 2*pair+1)
            pid = nc.sync.partition_id()
            ROWBLK = 128 * (KD * VCH // 4)    # f32 elements per rank block
            for g in range(G):
                for t in range(1, N_CORES):
                    if t == 7:
                        f, half = 12 + g // 2, g % 2
                    else:
                        f, half = 3 * g + (t - 1) // 2, (t - 1) % 2
                    r = (pid + t) % N_CORES
                    blk0 = vgath[g][0:128, :].rearrange(
                        "p (k c) -> p k c", k=KD)
                    for k2 in range(KD2):
                        s_ap = blk0[:, 2 * k2:2 * k2 + 2, :]
                        dyn = bass_mod.AP(
                            tensor=s_ap.tensor,
                            offset=r * ROWBLK + s_ap.offset,
                            ap=s_ap.ap,
                            dep_tracking_offset=s_ap.offset)
                        dst = vTf[k2][:, f * 1024:(f + 1) * 1024].bitcast(
                            F32).rearrange("p (ko n) -> p ko n", ko=2)
                        vt_loads.append(nc.sync.dma_start(
                            out=dst[:, :, half * 64:half * 64 + 64],
                            in_=dyn))

            ring_groups = [vis_loads, aud_loads1, aud_loads2, vt_loads]
            prev = None
            for grp in ring_groups:
                if not grp:
                    continue
                if prev is not None:
                    for h in grp:
                        add_dep_helper(h.ins, prev.ins, sync=False,
                                       reason="sync-ring class order")
                prev = grp[-1]

            # ---- main loop ------------------------------------------------
            # mx col = nt*128 + b*32 + q (q = g*8 + t).  PSUM groups:
            #   L     : 1-bank, block 14+pair, q = (2p)*8, (2p+1)*8  [early]
            #   B12@g : 2-bank, blocks 3g, 3g+1, q = g*8 + 1..4
            #   B3@g  : 1-bank, block 3g+2, q = g*8 + 5..6
            #   T7    : 1-bank, block 12+pair, q = (2p)*8+7, (2p+1)*8+7
            first_mm = [True]
            first_rd = [True]
            mx4 = mx[:].rearrange("p (nt b q) -> p nt b q", nt=NA // 128,
                                  b=BL)

            def mm_group(blocks, out_ap, nbank, b, nt):
                lcol = (b * (NA // 128) + nt) * 128
                pbig = ps.tile([128, 512 * nbank], F32, tag="mm",
                               name="pbig", bufs=2)
                for k2 in range(KD2):
                    lhs3 = aTf[k2][:].rearrange(
                        "p (ko m) -> p ko m", ko=2)[:, :, lcol:lcol + 128]
                    for s, f in enumerate(blocks):
                        rhs3 = vTf[k2][:].rearrange(
                            "p (f ko n) -> p f ko n", ko=2, n=512)[:, f]
                        mm = nc.tensor.matmul(
                            pbig[:, s * 512:(s + 1) * 512],
                            lhsT=lhs3, rhs=rhs3,
                            start=(k2 == 0), stop=(k2 == KD2 - 1),
                            perf_mode=mybir.MatmulPerfMode.DoubleRow)
                        if first_mm[0]:
                            # local groups may start as soon as the first
                            # audio batch is transposed
                            add_dep_helper(mm.ins, aud_recs[0].tr_last.ins,
                                           sync=False,
                                           reason="pe prep before main")
                            first_mm[0] = False
                rd = nc.vector.tensor_reduce(
                    out=out_ap,
                    in_=pbig[:].rearrange("p (j m) -> p j m", j=2 * nbank),
                    axis=AX.X, op=ALU.max)
                if first_rd[0]:
                    add_dep_helper(rd.ins, aud_recs[0].cp_last.ins,
                                   sync=False, reason="dve prep before main")
                    first_rd[0] = False

            pc128 = ps.tile([1, 128], F32, tag="pc", name="pc128", bufs=1)
            pcmm = [0]

            def pc_accum(g):
                for nt in range(NA // 128):
                    rhs = mx4[:, nt, :, g * 8:g * 8 + 8]
                    o = pc128[:].rearrange(
                        "p (b q) -> p b q", b=BL)[:, :, g * 8:g * 8 + 8]
                    nc.tensor.matmul(
                        o, lhsT=ones[:], rhs=rhs,
                        start=(pcmm[0] == 0),
                        stop=(pcmm[0] == G * (NA // 128) - 1))
                    pcmm[0] += 1

            # local groups first (b-major so batch-1 clips go earliest)
            for b in range(BL):
                for pair in range(2):
                    for nt in range(NA // 128):
                        out_ap = mx4[:, nt, b, :].rearrange(
                            "p (g q) -> p g q", g=G)[:, 2 * pair:
                                                     2 * pair + 2, 0:1]
                        mm_group([14 + pair], out_ap, 1, b, nt)
            # chunk-gated groups in gather-arrival order
            for g in range(G):
                for b in range(BL):
                    for nt in range(NA // 128):
                        mm_group([3 * g, 3 * g + 1],
                                 mx4[:, nt, b, g * 8 + 1:g * 8 + 5],
                                 2, b, nt)
                        mm_group([3 * g + 2],
                                 mx4[:, nt, b, g * 8 + 5:g * 8 + 7],
                                 1, b, nt)
                if g % 2 == 1:
                    pair = g // 2
                    for b in range(BL):
                        for nt in range(NA // 128):
                            out_ap = mx4[:, nt, b, :].rearrange(
                                "p (g2 q) -> p g2 q",
                                g2=G)[:, 2 * pair:2 * pair + 2, 7:8]
                            mm_group([12 + pair], out_ap, 1, b, nt)
                    pc_accum(2 * pair)
                    pc_accum(2 * pair + 1)

            # ---- tail: local softmax partials + one tiny AllReduce --------
            # clip[b, pos] = SC * pc128[b*32+pos]; |clip| <= ~3 so exp is
            # safe unstabilized in f32
            expm = wp.tile([1, 128], F32, tag="expm")
            nc.scalar.activation(expm[:], pc128[:], ACT.Exp, scale=SC)
            es = wp.tile([1, 4], F32, tag="es")
            nc.vector.tensor_reduce(
                out=es[:], in_=expm[:].rearrange("p (b q) -> p b q", b=4),
                axis=AX.X, op=ALU.add)
            lnes = wp.tile([1, 4], F32, tag="lnes")
            s1ln = wp.tile([1, 1], F32, tag="s1ln")
            nc.scalar.activation(lnes[:], es[:], ACT.Ln, accum_out=s1ln[:])
            dsc = wp.tile([1, 128], F32, tag="dsc")
            nc.vector.tensor_mul(dsc[:], pc128[:], dmask[:])
            d1 = wp.tile([1, 1], F32, tag="d1")
            nc.vector.reduce_sum(out=d1[:], in_=dsc[:], axis=AX.X)
            # E by local position via outer-product accumulation (puts E on
            # partitions), then map to global clip order: E_glob = E @ perm
            ecol = ps.tile([32, 1], F32, tag="pc", name="ecol", bufs=1)
            for b in range(BL):
                nc.tensor.matmul(ecol[:],
                                 lhsT=expm[0:1, b * 32:(b + 1) * 32],
                                 rhs=ones[0:1, 0:1],
                                 start=(b == 0), stop=(b == BL - 1))
            ecs = wp.tile([32, 1], F32, tag="ecs")
            nc.vector.tensor_copy(ecs[:], ecol[:])
            egl = ps.tile([1, 32], F32, tag="pc", name="egl", bufs=1)
            nc.tensor.matmul(egl[:], lhsT=ecs[:], rhs=perm[:],
                             start=True, stop=True)
            nc.vector.tensor_copy(vec[0:1, 0:32], egl[:])
            # w = (0.5*s1ln - d1) / B
            w0 = wp.tile([1, 1], F32, tag="w0")
            nc.vector.scalar_tensor_tensor(
                out=w0[:], in0=s1ln[:], scalar=0.5, in1=d1[:],
                op0=ALU.mult, op1=ALU.subtract)
            nc.scalar.mul(vec[0:1, 32:33], w0[:], 1.0 / B)

            ar_in = dp.tile([1, 64], F32, tag="ar_in", name="ar_in")
            nc.scalar.dma_start(out=ar_in[:], in_=vec[:])
            ar_out = dp.tile([1, 64], F32, tag="ar_out", name="ar_out",
                             addr_space="Shared")
            nc.gpsimd.collective_compute(
                "AllReduce", ALU.add, replica_groups=rg,
                ins=[ar_in[:, :].opt()], outs=[ar_out[:, :].opt()])

            rvec = wp.tile([1, 64], F32, tag="rvec")
            nc.sync.dma_start(out=rvec[:], in_=ar_out[:])
            lnE = wp.tile([1, 32], F32, tag="lnE")
            lnsum = wp.tile([1, 1], F32, tag="lnsum")
            nc.scalar.activation(lnE[:], rvec[0:1, 0:32], ACT.Ln,
                                 accum_out=lnsum[:])
            res = wp.tile([1, 1], F32, tag="res")
            nc.vector.scalar_tensor_tensor(
                out=res[:], in0=lnsum[:], scalar=0.5 / B,
                in1=rvec[0:1, 32:33], op0=ALU.mult, op1=ALU.add)
            nc.sync.dma_start(out=out[:], in_=res[:])

    nc.finalize()
    return nc


def _diag_mask(core):
    m = np.zeros((1, 128), dtype=np.float32)
    for b in range(BL):
        c = BL * core + b
        m[0, b * 32 + pos_of_clip(c, core)] = SC
    return m


def _perm(core):
    # E_glob[j] = sum_q E_loc[q] * PM[q, j]; j indexes global clip id
    m = np.zeros((32, 32), dtype=np.float32)
    for c in range(B):
        m[pos_of_clip(c, core), c] = 1.0
    return m


_NC_CACHE = None


def kernel(audio_feats: np.ndarray, visual_feats: np.ndarray) -> np.ndarray:
    from concourse.bass_utils import run_bass_kernel_spmd

    global _NC_CACHE
    if _NC_CACHE is None:
        _NC_CACHE = build()
    nc = _NC_CACHE

    audio = np.ascontiguousarray(audio_feats, dtype=np.float32)
    visual = np.ascontiguousarray(visual_feats, dtype=np.float32)
    in_maps = []
    for i in range(N_CORES):
        in_maps.append({
            "audio": audio[i * BL:(i + 1) * BL].reshape(AROWS, D),
            "visual": visual[i * BL:(i + 1) * BL].reshape(VROWS, D),
            "dmask": _diag_mask(i),
            "perm": _perm(i),
        })
    res = run_bass_kernel_spmd(nc, in_maps, core_ids=list(range(N_CORES)))
    val = res.results[0]["out"][0, 0]
    return np.asarray(val, dtype=np.float32)


if __name__ == "__main__":
    rng = np.random.default_rng(0)
    a = rng.standard_normal((B, NA, D)).astype(np.float32)
    v = rng.standard_normal((B, NV, D)).astype(np.float32)
    print(kernel(a, v))


# revision 16
# speedup vs baseline: 1.1372x; 1.1372x over previous
"""Distributed Trainium2 kernel for the audio-visual contrastive loss.

Math (reference):
    a = l2norm(audio)  (B=32, Na=512, D=768)
    v = l2norm(visual) (B=32, Nv=256, D=768)
    token_sims[b,c,n,m] = (a[b,n] . v[c,m]) / T
    clip_sims = mean_n max_m token_sims          (B, B)
    loss = mean_b -0.5*(log_softmax(clip)[b,b] + log_softmax(clip.T)[b,b])

Distribution over 8 NeuronCores:
    - audio batch is sharded 4 clips/core; visual batch is sharded 4 clips/core
      for the (normalize + transpose) prep, then AllGather'd as fp8 (d-major)
      in G=4 chunks (1 clip/core each) so the collectives pipeline with the
      main-loop matmuls.
    - RING-RELATIVE consumption: mx position q = g*8 + t where t is the ring
      step (t=0 = own clip).  Own clips come straight from local SBUF, so 96
      of the 768 matmuls run during the ~55us first-collective staging
      window; the other ranks are read from the gather output with a
      partition-id-relative dynamic row offset.
    - the loss is finished with per-core partial softmax stats and a single
      tiny AllReduce; per-core diag-mask and position-permutation inputs
      absorb all rank-dependence.
"""

import sys

for _p in ("/opt/trn_rl_repo",):
    if _p not in sys.path:
        sys.path.insert(0, _p)

import numpy as np

import concourse.bacc as bacc
import concourse.bass as bass_mod
import concourse.mybir as mybir
import concourse.tile as tile
from concourse.tile_rust import add_dep_helper

N_CORES = 8
B = 32
NA = 512
NV = 256
D = 768
TEMPERATURE = 0.1
BL = B // N_CORES            # 4 clips per core
AROWS = BL * NA              # 2048 audio rows per core
VROWS = BL * NV              # 1024 visual rows per core
KD = D // 128                # 6 contraction chunks
KD2 = KD // 2                # 3 DoubleRow chunk-pairs
NT_A = AROWS // 128          # 16 audio row-tiles
NT_V = VROWS // 128          # 8 visual row-tiles
G = 4                        # visual AllGather chunks (1 clip/core each)
VCH = VROWS // G             # 256 visual rows per chunk per core

F32 = mybir.dt.float32
BF16 = mybir.dt.bfloat16
FP8 = mybir.dt.float8e4
AX = mybir.AxisListType
ALU = mybir.AluOpType
ACT = mybir.ActivationFunctionType
SCL = 16.0                   # fp8 pre-scale (folded into the norm rsqrt)
SC = 1.0 / (NA * TEMPERATURE * SCL * SCL)   # psum-count -> clip_sims scale


def pos_of_clip(c, core):
    # ring-relative positions: clip c = 4*r + g sits at q = g*8 + t where
    # t = (r - core) mod 8 is the ring step (t=0 is the core's own clip).
    g = c % G
    r = c // BL
    t = (r - core) % N_CORES
    return g * 8 + t


def build():
    nc = bacc.Bacc("TRN2", target_bir_lowering=False, debug=False,
                   num_devices=N_CORES)
    a_in = nc.declare_dram_parameter("audio", [AROWS, D], F32, isOutput=False)
    v_in = nc.declare_dram_parameter("visual", [VROWS, D], F32,
                                     isOutput=False)
    dmask_in = nc.declare_dram_parameter("dmask", [1, 128], F32,
                                         isOutput=False)
    perm_in = nc.declare_dram_parameter("perm", [32, 32], F32,
                                        isOutput=False)
    out = nc.declare_dram_parameter("out", [1, 1], F32, isOutput=True)
    ident_dram = nc.inline_tensor(np.eye(128, dtype=np.float32), name="ident")
    rg = [list(range(N_CORES))]

    with tile.TileContext(nc) as tc:
        with (
            tc.tile_pool(name="persist", bufs=1) as pp,
            tc.tile_pool(name="work", bufs=3) as wp,
            tc.tile_pool(name="ps", bufs=2, space="PSUM") as ps,
            tc.tile_pool(name="dram", bufs=1, space="DRAM") as dp,
        ):
            # ---- constants ------------------------------------------------
            ident_f32 = pp.tile([128, 128], F32, tag="identf")
            nc.sync.dma_start(out=ident_f32[:], in_=ident_dram[:])
            ident_bf = pp.tile([128, 128], BF16, tag="identb")
            nc.scalar.copy(ident_bf[:], ident_f32[:])
            ones = pp.tile([128, 1], F32, tag="ones")
            nc.gpsimd.memset(ones[:], 1.0)
            # warm the ACT Ln/Exp tables off the critical path
            wrma = wp.tile([1, 1], F32, tag="wrma")
            nc.vector.memset(wrma[:], 1.0)
            wrmb = wp.tile([1, 1], F32, tag="wrmb")
            nc.scalar.activation(wrmb[:], wrma[:], ACT.Exp)
            wrmc = wp.tile([1, 1], F32, tag="wrmc")
            nc.scalar.activation(wrmc[:], wrmb[:], ACT.Ln)
            vec = pp.tile([1, 64], F32, tag="vec")
            nc.vector.memset(vec[:], 0.0)

            # ---- persistent tensors ---------------------------------------
            VTW = N_CORES * VROWS        # 8192 vT columns per d-chunk
            aTf = [pp.tile([128, 2 * AROWS], FP8, tag=f"aT8{k2}",
                           name=f"aT8{k2}") for k2 in range(KD2)]
            vstall = pp.tile([128, KD * VROWS], FP8, tag="vstall")
            vTf = [pp.tile([128, 2 * VTW], FP8, tag=f"vT8{k2}",
                           name=f"vT8{k2}") for k2 in range(KD2)]
            mx = pp.tile([128, 512], F32, tag="mx")

            # ---- row-tile prep --------------------------------------------
            class BatchRec:
                pass

            def prep_batch(src, t0, nb, dst_of_k, load_group):
                rec = BatchRec()
                raws = []
                ssb = wp.tile([128, nb], F32, tag="ssb", name="ssb", bufs=2)
                for j in range(nb):
                    t = t0 + j
                    raw = wp.tile([128, D], F32, tag="raw", name="raw",
                                  bufs=8)
                    load_group.append(
                        nc.sync.dma_start(out=raw[:],
                                          in_=src[t * 128:(t + 1) * 128, :]))
                    sq = nc.scalar.activation(
                        wp.tile([128, D], F32, tag="sqs", name="sqs",
                                bufs=2)[:],
                        raw[:], ACT.Square, accum_out=ssb[:, j:j + 1])
                    if j == 0:
                        rec.sq_first = sq
                    raws.append(raw)
                nrm = wp.tile([128, nb], F32, tag="nrm", name="nrm", bufs=2)
                nc.scalar.activation(nrm[:], ssb[:], ACT.Sqrt,
                                     scale=1.0 / (SCL * SCL))
                rnb = wp.tile([128, nb], F32, tag="rnb", name="rnb", bufs=2)
                nc.vector.reciprocal(rnb[:], nrm[:])
                nbfs = []
                for j in range(nb):
                    nbf = wp.tile([128, D], BF16, tag="nbf", name="nbf",
                                  bufs=6)
                    rec.cast_last = nc.scalar.activation(
                        nbf[:], raws[j][:], ACT.Copy, bias=0.0,
                        scale=rnb[:, j:j + 1])
                    nbfs.append(nbf)
                rec.tr_first = rec.tr_last = None
                rec.cp_first = rec.cp_last = None
                for j in range(nb):
                    for k in range(KD):
                        ptb = ps.tile([128, 128], BF16, tag="pt", name="ptb",
                                      bufs=3)
                        tr = nc.tensor.transpose(
                            ptb[:], nbfs[j][:, 128 * k:128 * (k + 1)],
                            ident_bf[:])
                        dst_tile, col = dst_of_k(t0, k)
                        cp = nc.vector.tensor_copy(
                            dst_tile[:, col + j * 128:col + (j + 1) * 128],
                            ptb[:])
                        if rec.tr_first is None:
                            rec.tr_first, rec.cp_first = tr, cp
                        rec.tr_last, rec.cp_last = tr, cp
                return rec

            # ---- visual prep + bounce + chunked AllGather -----------------
            vis_loads, aud_loads1, aud_loads2 = [], [], []
            vt_loads = []
            vgath = []
            vis_recs, aud_recs = [], []
            nbv = NT_V // G              # 2 tiles per chunk
            vst3 = vstall[:].rearrange("p (k c) -> p k c", k=KD)
            for g in range(G):
                vis_recs.append(prep_batch(
                    v_in, g * nbv, nbv,
                    lambda t0, k: (vstall, k * VROWS + (t0 // nbv) * VCH),
                    vis_loads))
                vb = dp.tile([128, KD * VCH // 4], F32, tag=f"vb{g}",
                             name=f"vb{g}")
                nc.scalar.dma_start(
                    out=vb[:, :].rearrange("p (k c) -> p k c", k=KD),
                    in_=vst3[:, :, g * VCH:(g + 1) * VCH].bitcast(F32))
                vg = dp.tile([N_CORES * 128, KD * VCH // 4], F32,
                             tag=f"vg{g}", name=f"vg{g}",
                             addr_space="Shared")
                nc.gpsimd.collective_compute(
                    "AllGather", ALU.bypass, replica_groups=rg,
                    ins=[vb[:, :].opt()], outs=[vg[:, :].opt()])
                vgath.append(vg)

            dmask = pp.tile([1, 128], F32, tag="dmask")
            vis_loads.append(nc.sync.dma_start(out=dmask[:], in_=dmask_in[:]))
            perm = pp.tile([32, 32], F32, tag="perm")
            vis_loads.append(nc.sync.dma_start(out=perm[:], in_=perm_in[:]))

            # ---- own clips: vstall -> vTf local blocks (DVE engine ops, ---
            # no DMA ring involved so the scheduler cannot push them behind
            # the gathers).  Block 14+pair holds (t=0 of chunks 2p, 2p+1).
            loc_first, loc_last = None, None
            for pair in range(2):
                fL = 14 + pair
                for k2 in range(KD2):
                    dst = vTf[k2][:, fL * 1024:(fL + 1) * 1024].bitcast(
                        F32).rearrange("p (ko h n) -> p ko h n", ko=2, h=2)
                    src = vst3[:, 2 * k2:2 * k2 + 2,
                               2 * pair * VCH:
                               (2 * pair + 2) * VCH].bitcast(F32).rearrange(
                        "p ko (h n) -> p ko h n", h=2)
                    cp = nc.vector.tensor_copy(dst, src)
                    if loc_first is None:
                        loc_first = cp
                    loc_last = cp

            # ---- audio prep ----------------------------------------------
            for t0 in range(0, NT_A, 8):
                aud_recs.append(prep_batch(
                    a_in, t0, 8,
                    lambda t0_, k: (aTf[k // 2],
                                    (k % 2) * AROWS + t0_ * 128),
                    aud_loads1 if t0 == 0 else aud_loads2))

            # ---- scheduler pins: visual prep, then local copies, then -----
            # audio prep, per engine.  Without these the scheduler hoists
            # audio prep ahead of the visual chain and the bounces (hence
            # the gathers) slip by ~20us.
            add_dep_helper(aud_recs[0].sq_first.ins,
                           vis_recs[-1].cast_last.ins, sync=False,
                           reason="act: visual prep first")
            add_dep_helper(aud_recs[0].tr_first.ins,
                           vis_recs[-1].tr_last.ins, sync=False,
                           reason="pe: visual prep first")
            add_dep_helper(loc_first.ins, vis_recs[-1].cp_last.ins,
                           sync=False, reason="dve: local after visual")
            add_dep_helper(aud_recs[0].cp_first.ins, loc_last.ins,
                           sync=False, reason="dve: audio after local")

            # ---- other ranks: gather output -> vTf, ring-relative ---------
            # dynamic row offset r = (pid+t)%8 selects the rank block.
            #   f = 3*g + u      (u=0..2): chunk g, ring steps (2u+1, 2u+2)
            #   f = 12 + pair    : ring step 7 of chunks (2*pair, 2*pair+1)
            pid = nc.sync.partition_id()
            pid_act = nc.scalar.partition_id()
            ROWBLK = 128 * (KD * VCH // 4)    # f32 elements per rank block
            for g in range(G):
                for t in range(1, N_CORES):
                    if t == 7:
                        f, half = 12 + g // 2, g % 2
                    else:
                        f, half = 3 * g + (t - 1) // 2, (t - 1) % 2
                    r = (pid + t) % N_CORES
                    r_act = (pid_act + t) % N_CORES
                    blk0 = vgath[g][0:128, :].rearrange(
                        "p (k c) -> p k c", k=KD)
                    for k2 in range(KD2):
                        s_ap = blk0[:, 2 * k2:2 * k2 + 2, :]
                        roff = r if k2 < 2 else r_act
                        dyn = bass_mod.AP(
                            tensor=s_ap.tensor,
                            offset=roff * ROWBLK + s_ap.offset,
                            ap=s_ap.ap,
                            dep_tracking_offset=s_ap.offset)
                        dst = vTf[k2][:, f * 1024:(f + 1) * 1024].bitcast(
                            F32).rearrange("p (ko n) -> p ko n", ko=2)
                        if k2 < 2:
                            vt_loads.append(nc.sync.dma_start(
                                out=dst[:, :, half * 64:half * 64 + 64],
                                in_=dyn))
                        else:
                            # scalar ring carries a third of the loads so the
                            # two sequencers issue in parallel; pinned after
                            # the audio prep so the ACT queue is not blocked
                            # by the gather wait mid-prep
                            h2 = nc.scalar.dma_start(
                                out=dst[:, :, half * 64:half * 64 + 64],
                                in_=dyn)
                            add_dep_helper(h2.ins,
                                           aud_recs[-1].cast_last.ins,
                                           sync=False,
                                           reason="act vt after prep")

            ring_groups = [vis_loads, aud_loads1, aud_loads2, vt_loads]
            prev = None
            for grp in ring_groups:
                if not grp:
                    continue
                if prev is not None:
                    for h in grp:
                        add_dep_helper(h.ins, prev.ins, sync=False,
                                       reason="sync-ring class order")
                prev = grp[-1]

            # ---- main loop ------------------------------------------------
            # mx col = nt*128 + b*32 + q (q = g*8 + t).  PSUM groups:
            #   L     : 1-bank, block 14+pair, q = (2p)*8, (2p+1)*8  [early]
            #   B12@g : 2-bank, blocks 3g, 3g+1, q = g*8 + 1..4
            #   B3@g  : 1-bank, block 3g+2, q = g*8 + 5..6
            #   T7    : 1-bank, block 12+pair, q = (2p)*8+7, (2p+1)*8+7
            first_mm = [True]
            first_rd = [True]
            mx4 = mx[:].rearrange("p (nt b q) -> p nt b q", nt=NA // 128,
                                  b=BL)

            def mm_group(blocks, out_ap, nbank, b, nt):
                lcol = (b * (NA // 128) + nt) * 128
                pbig = ps.tile([128, 512 * nbank], F32, tag="mm",
                               name="pbig", bufs=2)
                for k2 in range(KD2):
                    lhs3 = aTf[k2][:].rearrange(
                        "p (ko m) -> p ko m", ko=2)[:, :, lcol:lcol + 128]
                    for s, f in enumerate(blocks):
                        rhs3 = vTf[k2][:].rearrange(
                            "p (f ko n) -> p f ko n", ko=2, n=512)[:, f]
                        mm = nc.tensor.matmul(
                            pbig[:, s * 512:(s + 1) * 512],
                            lhsT=lhs3, rhs=rhs3,
                            start=(k2 == 0), stop=(k2 == KD2 - 1),
                            perf_mode=mybir.MatmulPerfMode.DoubleRow)
                        if first_mm[0]:
                            # local groups may start as soon as the first
                            # audio batch is transposed
                            add_dep_helper(mm.ins, aud_recs[0].tr_last.ins,
                                           sync=False,
                                           reason="pe prep before main")
                            first_mm[0] = False
                rd = nc.vector.tensor_reduce(
                    out=out_ap,
                    in_=pbig[:].rearrange("p (j m) -> p j m", j=2 * nbank),
                    axis=AX.X, op=ALU.max)
                if first_rd[0]:
                    add_dep_helper(rd.ins, aud_recs[0].cp_last.ins,
                                   sync=False, reason="dve prep before main")
                    first_rd[0] = False

            pc128 = ps.tile([1, 128], F32, tag="pc", name="pc128", bufs=1)
            pcmm = [0]

            def pc_accum(g):
                for nt in range(NA // 128):
                    rhs = mx4[:, nt, :, g * 8:g * 8 + 8]
                    o = pc128[:].rearrange(
                        "p (b q) -> p b q", b=BL)[:, :, g * 8:g * 8 + 8]
                    nc.tensor.matmul(
                        o, lhsT=ones[:], rhs=rhs,
                        start=(pcmm[0] == 0),
                        stop=(pcmm[0] == G * (NA // 128) - 1))
                    pcmm[0] += 1

            # local groups first (both L blocks = one uniform 2-bank group;
            # all groups are 6-MM bursts like the proven v3 pipeline shape)
            for b in range(BL):
                for nt in range(NA // 128):
                    out_ap = mx4[:, nt, b, :].rearrange(
                        "p (g q) -> p g q", g=G)[:, :, 0:1]
                    mm_group([14, 15], out_ap, 2, b, nt)
            # chunk-gated groups in gather-arrival order; B3 pairs merge
            # across adjacent chunks, T7 after the last chunk
            for g in range(G):
                for b in range(BL):
                    for nt in range(NA // 128):
                        mm_group([3 * g, 3 * g + 1],
                                 mx4[:, nt, b, g * 8 + 1:g * 8 + 5],
                                 2, b, nt)
                if g % 2 == 1:
                    for b in range(BL):
                        for nt in range(NA // 128):
                            mm_group([3 * (g - 1) + 2, 3 * g + 2],
                                     mx4[:, nt, b, :].rearrange(
                                         "p (g2 q) -> p g2 q",
                                         g2=G)[:, g - 1:g + 1, 5:7],
                                     2, b, nt)
            for b in range(BL):
                for nt in range(NA // 128):
                    out_ap = mx4[:, nt, b, :].rearrange(
                        "p (g q) -> p g q", g=G)[:, :, 7:8]
                    mm_group([12, 13], out_ap, 2, b, nt)
            for g in range(G):
                pc_accum(g)

            # ---- tail: local softmax partials + one tiny AllReduce --------
            # clip[b, pos] = SC * pc128[b*32+pos]; |clip| <= ~3 so exp is
            # safe unstabilized in f32
            expm = wp.tile([1, 128], F32, tag="expm")
            nc.scalar.activation(expm[:], pc128[:], ACT.Exp, scale=SC)
            es = wp.tile([1, 4], F32, tag="es")
            nc.vector.tensor_reduce(
                out=es[:], in_=expm[:].rearrange("p (b q) -> p b q", b=4),
                axis=AX.X, op=ALU.add)
            lnes = wp.tile([1, 4], F32, tag="lnes")
            s1ln = wp.tile([1, 1], F32, tag="s1ln")
            nc.scalar.activation(lnes[:], es[:], ACT.Ln, accum_out=s1ln[:])
            dsc = wp.tile([1, 128], F32, tag="dsc")
            nc.vector.tensor_mul(dsc[:], pc128[:], dmask[:])
            d1 = wp.tile([1, 1], F32, tag="d1")
            nc.vector.reduce_sum(out=d1[:], in_=dsc[:], axis=AX.X)
            # E by local position via outer-product accumulation (puts E on
            # partitions), then map to global clip order: E_glob = E @ perm
            ecol = ps.tile([32, 1], F32, tag="pc", name="ecol", bufs=1)
            for b in range(BL):
                nc.tensor.matmul(ecol[:],
                                 lhsT=expm[0:1, b * 32:(b + 1) * 32],
                                 rhs=ones[0:1, 0:1],
                                 start=(b == 0), stop=(b == BL - 1))
            ecs = wp.tile([32, 1], F32, tag="ecs")
            nc.vector.tensor_copy(ecs[:], ecol[:])
            egl = ps.tile([1, 32], F32, tag="pc", name="egl", bufs=1)
            nc.tensor.matmul(egl[:], lhsT=ecs[:], rhs=perm[:],
                             start=True, stop=True)
            nc.vector.tensor_copy(vec[0:1, 0:32], egl[:])
            # w = (0.5*s1ln - d1) / B
            w0 = wp.tile([1, 1], F32, tag="w0")
            nc.vector.scalar_tensor_tensor(
                out=w0[:], in0=s1ln[:], scalar=0.5, in1=d1[:],
                op0=ALU.mult, op1=ALU.subtract)
            nc.scalar.mul(vec[0:1, 32:33], w0[:], 1.0 / B)

            ar_in = dp.tile([1, 64], F32, tag="ar_in", name="ar_in")
            nc.scalar.dma_start(out=ar_in[:], in_=vec[:])
            ar_out = dp.tile([1, 64], F32, tag="ar_out", name="ar_out",
                             addr_space="Shared")
            nc.gpsimd.collective_compute(
                "AllReduce", ALU.add, replica_groups=rg,
                ins=[ar_in[:, :].opt()], outs=[ar_out[:, :].opt()])

            rvec = wp.tile([1, 64], F32, tag="rvec")
            nc.sync.dma_start(out=rvec[:], in_=ar_out[:])
            lnE = wp.tile([1, 32], F32, tag="lnE")
            lnsum = wp.tile([1, 1], F32, tag="lnsum")
            nc.scalar.activation(lnE[:], rvec[0:1, 0:32], ACT.Ln,
                                 accum_out=lnsum[:])
            res = wp.tile([1, 1], F32, tag="res")
            nc.vector.scalar_tensor_tensor(
                out=res[:], in0=lnsum[:], scalar=0.5 / B,
                in1=rvec[0:1, 32:33], op0=ALU.mult, op1=ALU.add)
            nc.sync.dma_start(out=out[:], in_=res[:])

    nc.finalize()
    return nc


def _diag_mask(core):
    m = np.zeros((1, 128), dtype=np.float32)
    for b in range(BL):
        c = BL * core + b
        m[0, b * 32 + pos_of_clip(c, core)] = SC
    return m


def _perm(core):
    # E_glob[j] = sum_q E_loc[q] * PM[q, j]; j indexes global clip id
    m = np.zeros((32, 32), dtype=np.float32)
    for c in range(B):
        m[pos_of_clip(c, core), c] = 1.0
    return m


_NC_CACHE = None


def kernel(audio_feats: np.ndarray, visual_feats: np.ndarray) -> np.ndarray:
    from concourse.bass_utils import run_bass_kernel_spmd

    global _NC_CACHE
    if _NC_CACHE is None:
        _NC_CACHE = build()
    nc = _NC_CACHE

    audio = np.ascontiguousarray(audio_feats, dtype=np.float32)
    visual = np.ascontiguousarray(visual_feats, dtype=np.float32)
    in_maps = []
    for i in range(N_CORES):
        in_maps.append({
            "audio": audio[i * BL:(i + 1) * BL].reshape(AROWS, D),
            "visual": visual[i * BL:(i + 1) * BL].reshape(VROWS, D),
            "dmask": _diag_mask(i),
            "perm": _perm(i),
        })
    res = run_bass_kernel_spmd(nc, in_maps, core_ids=list(range(N_CORES)))
    val = res.results[0]["out"][0, 0]
    return np.asarray(val, dtype=np.float32)


if __name__ == "__main__":
    rng = np.random.default_rng(0)
    a = rng.standard_normal((B, NA, D)).astype(np.float32)
    v = rng.standard_normal((B, NV, D)).astype(np.float32)
    print(kernel(a, v))
